# revision 21
# baseline (speedup 1.0000x reference)
"""Longhorn SSM layer on 8 Trainium2 cores.

Sharding: core (b, j) with b in {0,1}, j in {0..3} handles batch b and
d_inner channel chunk [j*512, (j+1)*512).  The x_proj contraction needs all
d_inner channels, so partial x_dbl results are AllReduced across the 4 cores
of each batch (per time-chunk).  The final out_proj partials are summed on
the host.

Layout: time axis split into H=4 chunks of 512, software-pipelined so
in_proj / collective / out_proj hide under the DVE-bound selective scan.
The depthwise conv is folded into the in_proj matmul (4 host-prescaled
weight copies accumulated at shifted time offsets).  GpSimd is kept idle
during the scan: its SBUF port is shared with the DVE 2x modes and a
running GpSimd op fully blocks them.  All products run on DVE in bf16
(2x), a_t and all copies/activations on the Scalar engine, y = sum_n s*q
and the D*x skip term accumulate on the PE via identity/diagonal matmuls.
"""

import sys

if "/opt/trn_rl_repo" not in sys.path:
    sys.path.append("/opt/trn_rl_repo")

import numpy as np
import ml_dtypes

import concourse.bacc as bacc
import concourse.bass as bass
import concourse.tile as tile
from concourse import mybir
from concourse.bass_utils import run_bass_kernel_spmd

F32 = mybir.dt.float32
BF16 = mybir.dt.bfloat16
AL = mybir.AluOpType
AF = mybir.ActivationFunctionType

BF = ml_dtypes.bfloat16


def build_module(L, DM, DI, DCH, NST, DTR, num_devices, use_collective):
    NG = DCH // 128          # d-tiles per core
    NK = DM // 128           # K-tiles for in_proj
    NO = DM // 128           # out_proj output tiles
    H = 4                    # time chunks
    C = L // H
    NR = DTR + 2 * NST       # x_proj rows
    PL = L + 4               # padded time length (3 zeros + L + 1 zero)

    nc = bacc.Bacc(
        "TRN2",
        target_bir_lowering=False,
        debug=False,
        enable_asserts=False,
        num_devices=num_devices,
    )

    # ---- I/O -------------------------------------------------------------
    hT_d = nc.dram_tensor("hTp", [DM, PL], BF16, kind="ExternalInput")
    wx4_d = nc.dram_tensor("wx4", [128, 4 * NK * NG * 128], BF16,
                           kind="ExternalInput")
    wz_d = nc.dram_tensor("wz", [128, NK * NG * 128], BF16, kind="ExternalInput")
    wo_d = nc.dram_tensor("wo", [128, NG * NO * 128], BF16, kind="ExternalInput")
    dtw_d = nc.dram_tensor("dtw", [DTR, NG * 128], BF16, kind="ExternalInput")
    xpw_d = nc.dram_tensor("xpw", [128, NG * NR], BF16, kind="ExternalInput")
    dd_d = nc.dram_tensor("dd", [128, NG * 128], BF16, kind="ExternalInput")
    pvec_d = nc.dram_tensor("pvec", [128, NG * 7], F32, kind="ExternalInput")
    ones_d = nc.dram_tensor("ones16", [NST, 128], BF16, kind="ExternalInput")
    id_d = nc.dram_tensor("id128", [128, 128], BF16, kind="ExternalInput")
    outT_d = nc.dram_tensor("outT", [DM, L], F32, kind="ExternalOutput")

    # internal DRAM (per-chunk collective buffers + broadcast rows)
    ccin_d = [nc.dram_tensor(f"ccin{h}", [NR, C], F32, kind="Internal")
              for h in range(H)]
    ccout_d = [nc.dram_tensor(f"ccout{h}", [NR, C], F32, kind="Internal")
               for h in range(H)]
    kbd = nc.dram_tensor("kbd", [NST, L], BF16, kind="Internal")
    qbd = nc.dram_tensor("qbd", [NST, L], BF16, kind="Internal")
    kkbd = nc.dram_tensor("kkbd", [NST, L], BF16, kind="Internal")
    cwu_i = nc.dram_tensor("cwu_i", [8, 4], F32, kind="Internal")
    cwu_o = nc.dram_tensor("cwu_o", [8, 4], F32, kind="Internal")

    groups = [[0, 1, 2, 3], [4, 5, 6, 7]] if num_devices == 8 else [[0]]

    with tile.TileContext(nc) as tc:
        with (
            tc.tile_pool(name="const", bufs=1) as constp,
            tc.tile_pool(name="persist", bufs=1) as pp,
            tc.tile_pool(name="weights", bufs=1) as wp,
            tc.tile_pool(name="hsb", bufs=2) as hp,
            tc.tile_pool(name="rows", bufs=1) as rowp,
            tc.tile_pool(name="a2", bufs=1) as a2p,
            tc.tile_pool(name="bcast", bufs=3) as bcp,
            tc.tile_pool(name="scan", bufs=4) as scp,
            tc.tile_pool(name="scana", bufs=3) as scap,
            tc.tile_pool(name="drain", bufs=2) as drp,
            tc.tile_pool(name="odr", bufs=2) as odp,
            tc.tile_pool(name="psY", bufs=1, space="PSUM") as psYp,
            tc.tile_pool(name="ps", bufs=2, space="PSUM") as psp,
            tc.tile_pool(name="pdk", bufs=2, space="PSUM") as pdkp,
        ):
            # warmup collective: absorbs channel-setup latency off AR(0)'s
            # critical path while the weight DMAs stream in
            if use_collective:
                nc.gpsimd.collective_compute(
                    "AllReduce", AL.add, replica_groups=groups,
                    ins=[cwu_i.ap()], outs=[cwu_o.ap()])

            # chunk-0 activations first, then weights ordered so chunk 0's
            # in_proj can start ASAP
            hsb0 = hp.tile([128, NK, C + 3], BF16, name="hsb", tag="hsb")
            for k in range(NK):
                nc.sync.dma_start(
                    hsb0[:, k], hT_d.ap()[k * 128:(k + 1) * 128, 0:C + 3])
            wx_sb = wp.tile([128, 4, NK, NG, 128], BF16)
            wz_sb = wp.tile([128, NK, NG, 128], BF16)
            for g in range(NG):
                nc.sync.dma_start(
                    wx_sb[:, :, :, g, :],
                    wx4_d.ap().rearrange("p (j k g m) -> p j k g m",
                                         j=4, k=NK, g=NG)[:, :, :, g, :])
                nc.sync.dma_start(
                    wz_sb[:, :, g, :],
                    wz_d.ap().rearrange("p (k g m) -> p k g m",
                                        k=NK, g=NG)[:, :, g, :])
            pvec = constp.tile([128, NG, 7], F32)
            nc.sync.dma_start(pvec, pvec_d.ap().rearrange("p (g c) -> p g c", g=NG))
            xpw_sb = wp.tile([128, NG, NR], BF16)
            nc.sync.dma_start(
                xpw_sb, xpw_d.ap().rearrange("p (g r) -> p g r", g=NG))
            dtw_sb = wp.tile([DTR, NG, 128], BF16)
            nc.sync.dma_start(
                dtw_sb, dtw_d.ap().rearrange("p (g m) -> p g m", g=NG))
            ones_sb = constp.tile([NST, 128], BF16)
            nc.sync.dma_start(ones_sb, ones_d.ap())
            id_sb = constp.tile([128, 128], BF16)
            nc.sync.dma_start(id_sb, id_d.ap())
            dd_sb = constp.tile([128, NG, 128], BF16)
            nc.sync.dma_start(dd_sb, dd_d.ap().rearrange("p (g m) -> p g m", g=NG))
            wo_sb = wp.tile([128, NG, NO, 128], BF16)
            nc.sync.dma_start(
                wo_sb, wo_d.ap().rearrange("p (g o m) -> p g o m", g=NG, o=NO))

            # persistent per-core state (bf16, 16KB/partition each)
            xs = pp.tile([128, NG, L], BF16)      # silu(conv(x))
            dtvb = pp.tile([128, NG, L], BF16)
            ub = pp.tile([128, NG, L], BF16)      # x*dtv
            silz = pp.tile([128, NG, L], BF16)    # silu(z)
            ygb = pp.tile([128, NG, L], BF16)     # gated y
            sc = pp.tile([128, NG * NST], F32)    # scan carry

            Y = psYp.tile([128, NG * C], F32)     # 4 PSUM banks

            # ---------------- emission helpers ---------------------------
            def emit_inproj_g(h, hsb, g):
                R = slice(h * C, (h + 1) * C)
                px = psp.tile([128, C], F32, name="px", tag="ps")
                for j in range(4):
                    for k in range(NK):
                        nc.tensor.matmul(px, wx_sb[:, j, k, g, :],
                                         hsb[:, k, j:j + C],
                                         start=(j == 0 and k == 0),
                                         stop=(j == 3 and k == NK - 1))
                nc.scalar.activation(xs[:, g, R], px, AF.Silu,
                                     bias=pvec[:, g, 6:7], scale=1.0)
                pz = psp.tile([128, C], F32, name="pz", tag="ps")
                for k in range(NK):
                    nc.tensor.matmul(pz, wz_sb[:, k, g, :], hsb[:, k, 3:3 + C],
                                     start=(k == 0), stop=(k == NK - 1))
                nc.scalar.activation(silz[:, g, R], pz, AF.Silu)

            def emit_xproj(h):
                R = slice(h * C, (h + 1) * C)
                pX = psp.tile([128, C], F32, name="pX", tag="ps")
                for g in range(NG):
                    nc.tensor.matmul(pX[0:NR, :], xpw_sb[:, g, :], xs[:, g, R],
                                     start=(g == 0), stop=(g == NG - 1))
                xdp = rowp.tile([NR, C], F32, name="xdp", tag="xdp")
                nc.scalar.copy(xdp, pX[0:NR, :])
                nc.sync.dma_start(ccin_d[h].ap(), xdp)

            def emit_collective(h):
                if use_collective:
                    nc.gpsimd.collective_compute(
                        "AllReduce", AL.add, replica_groups=groups,
                        ins=[ccin_d[h].ap()], outs=[ccout_d[h].ap()])
                else:
                    nc.sync.dma_start(ccout_d[h].ap(), ccin_d[h].ap())

            def emit_a2(h):
                R = slice(h * C, (h + 1) * C)
                dtl = rowp.tile([DTR, C], F32, name="dtl", tag="dtl")
                nc.sync.dma_start(dtl, ccout_d[h].ap()[0:DTR, :])
                krow = rowp.tile([NST, C], F32, name="krow", tag="krow")
                nc.sync.dma_start(krow, ccout_d[h].ap()[DTR:DTR + NST, :])
                qrow = rowp.tile([NST, C], F32, name="qrow", tag="qrow")
                nc.sync.dma_start(qrow, ccout_d[h].ap()[DTR + NST:NR, :])

                dtl_b = rowp.tile([DTR, C], BF16, name="dtlb", tag="dtlb")
                nc.scalar.copy(dtl_b, dtl)
                kkb = rowp.tile([NST, C], BF16, name="kkb", tag="kkb")
                nc.scalar.activation(kkb, krow, AF.Square)
                kb16 = rowp.tile([NST, C], BF16, name="kb16", tag="kb16")
                nc.scalar.copy(kb16, krow)
                qb16 = rowp.tile([NST, C], BF16, name="qb16", tag="qb16")
                nc.scalar.copy(qb16, qrow)
                nc.sync.dma_start(kbd.ap()[:, R], kb16)
                nc.sync.dma_start(qbd.ap()[:, R], qb16)
                nc.sync.dma_start(kkbd.ap()[:, R], kkb)

                psK = pdkp.tile([128, C], F32, name="psK", tag="pdk")
                nc.tensor.matmul(psK, ones_sb, kkb, start=True, stop=True)
                skt = a2p.tile([128, C], F32, name="skt", tag="skt")
                nc.scalar.activation(skt, psK, AF.Identity, bias=1.0)
                for g in range(NG):
                    pD = pdkp.tile([128, C], F32, name="pD", tag="pdk")
                    nc.tensor.matmul(pD, dtw_sb[:, g, :], dtl_b,
                                     start=True, stop=True)
                    eg = a2p.tile([128, C], F32, name="eg", tag="eg")
                    nc.scalar.activation(eg, pD, AF.Exp,
                                         bias=pvec[:, g, 4:5], scale=-1.0)
                    den = a2p.tile([128, C], F32, name="den", tag="den")
                    nc.vector.tensor_tensor(den, eg, skt, op=AL.add)
                    dtv = a2p.tile([128, C], F32, name="dtvf", tag="dtvf")
                    nc.vector.reciprocal_approx_fast(out=dtv, in_=den)
                    nc.scalar.copy(dtvb[:, g, R], dtv)
                    nc.vector.tensor_tensor(ub[:, g, R], xs[:, g, R],
                                            dtvb[:, g, R], op=AL.mult)

            def emit_bcast(h, n):
                R = slice(h * C, (h + 1) * C)
                kkb_t = bcp.tile([128, C], BF16, name="kkb_t", tag="kkb")
                nc.sync.dma_start(
                    kkb_t, kkbd.ap()[n:n + 1, R].broadcast_to([128, C]))
                kb_t = bcp.tile([128, C], BF16, name="kb_t", tag="kb")
                nc.sync.dma_start(
                    kb_t, kbd.ap()[n:n + 1, R].broadcast_to([128, C]))
                qb_t = bcp.tile([128, C], BF16, name="qb_t", tag="qb")
                nc.sync.dma_start(
                    qb_t, qbd.ap()[n:n + 1, R].broadcast_to([128, C]))
                return kkb_t, kb_t, qb_t

            def emit_products(h, n, g, kkb_t, kb_t):
                # stage 1 of a scan iteration: c, a, b (lookahead-emitted so
                # ACT's a_t computes during the previous scan instruction)
                R = slice(h * C, (h + 1) * C)
                c_t = scp.tile([128, C], BF16, name="c_t", tag="c")
                nc.vector.tensor_tensor(c_t, dtvb[:, g, R], kkb_t, op=AL.mult)
                a_t = scap.tile([128, C], F32, name="a_t", tag="a")
                nc.scalar.activation(a_t, c_t, AF.Identity, bias=1.0, scale=-1.0)
                b_t = scp.tile([128, C], BF16, name="b_t", tag="b")
                nc.vector.tensor_tensor(b_t, ub[:, g, R], kb_t, op=AL.mult)
                return a_t, b_t

            def emit_scan_tail(h, n, g, a_t, b_t, qb_t):
                col = g * NST + n
                s_t = scp.tile([128, C], BF16, name="s_t", tag="s")
                init = 0.0 if h == 0 else sc[:, col:col + 1]
                nc.vector.tensor_tensor_scan(
                    s_t, a_t, b_t, init, op0=AL.mult, op1=AL.add)
                if h < H - 1:
                    nc.scalar.copy(sc[:, col:col + 1], s_t[:, C - 1:C])
                p_t = scp.tile([128, C], BF16, name="p_t", tag="p")
                nc.vector.tensor_tensor(p_t, s_t, qb_t, op=AL.mult)
                nc.tensor.matmul(Y[:, g * C:(g + 1) * C], id_sb, p_t,
                                 start=(n == 0), stop=False)

            def emit_drain(h):
                R = slice(h * C, (h + 1) * C)
                for g in range(NG):
                    # D*x skip term closes the Y accumulation group
                    nc.tensor.matmul(Y[:, g * C:(g + 1) * C], dd_sb[:, g, :],
                                     xs[:, g, R], start=False, stop=True)
                    yb = drp.tile([128, C], BF16, name="yb", tag="yb")
                    nc.scalar.copy(yb, Y[:, g * C:(g + 1) * C])
                    nc.vector.tensor_tensor(ygb[:, g, R], yb, silz[:, g, R],
                                            op=AL.mult)

            def emit_outproj(h):
                R = slice(h * C, (h + 1) * C)
                for o in range(NO):
                    po = psp.tile([128, C], F32, name="po", tag="ps")
                    for g in range(NG):
                        nc.tensor.matmul(po, wo_sb[:, g, o, :], ygb[:, g, R],
                                         start=(g == 0), stop=(g == NG - 1))
                    ot = odp.tile([128, C], F32, name="ot", tag="ot")
                    nc.scalar.copy(ot, po)
                    nc.sync.dma_start(outT_d.ap()[o * 128:(o + 1) * 128, R], ot)

            def load_hsb(h):
                t = hp.tile([128, NK, C + 3], BF16, name="hsb", tag="hsb")
                for k in range(NK):
                    nc.sync.dma_start(
                        t[:, k], hT_d.ap()[k * 128:(k + 1) * 128,
                                           h * C:h * C + C + 3])
                return t

            # ---------------- chunk 0 prologue ---------------------------
            for g in range(NG):
                emit_inproj_g(0, hsb0, g)
            emit_xproj(0)
            emit_collective(0)
            emit_a2(0)

            # ---------------- pipelined chunks ----------------------------
            # one-iteration lookahead: products (c/a/b) for iter i+1 are
            # emitted before scan_tail(i), so the ACT a_t overlaps the scan
            prev = None
            hsb_n = [None] * (H + 1)
            bc = None
            for h in range(H):
                for n in range(NST):
                    bc_new = emit_bcast(h, n)
                    for g in range(NG):
                        if g == 0:
                            bc = bc_new
                        a_t, b_t = emit_products(h, n, g, bc[0], bc[1])
                        if prev is not None:
                            emit_scan_tail(*prev)
                            ph, pn, pg = prev[0], prev[1], prev[2]
                            if pn == NST - 1 and pg == NG - 1:
                                emit_drain(ph)
                                emit_outproj(ph)
                        prev = (h, n, g, a_t, b_t, bc[2])
                    # interleave next chunk's prep into the scan
                    nxt = h + 1
                    if nxt < H:
                        if n == 0:
                            hsb_n[nxt] = load_hsb(nxt)
                        elif 1 <= n <= 4:
                            emit_inproj_g(nxt, hsb_n[nxt], n - 1)
                        elif n == 5:
                            emit_xproj(nxt)
                        elif n == 6:
                            emit_collective(nxt)
                        elif n == 11:
                            emit_a2(nxt)
            emit_scan_tail(*prev)
            emit_drain(H - 1)
            emit_outproj(H - 1)

    nc.compile()
    return nc


# ----------------------------------------------------------------------------
# host-side packing
# ----------------------------------------------------------------------------

def pack_core_inputs(inputs, b, j, L, DM, DI, DCH, NST, DTR):
    NG = DCH // 128
    NK = DM // 128
    NO = DM // 128
    NR = DTR + 2 * NST
    PL = L + 4
    ch = slice(j * DCH, (j + 1) * DCH)

    h = np.ascontiguousarray(np.asarray(inputs["hidden_states"], np.float32))
    ipw = np.asarray(inputs["in_proj_w"], np.float32)
    cw = np.asarray(inputs["conv_w"], np.float32).reshape(DI, 4)
    cb = np.asarray(inputs["conv_b"], np.float32)
    xpw = np.asarray(inputs["x_proj_w"], np.float32)
    dtw = np.asarray(inputs["dt_head_w"], np.float32)
    dtb = np.asarray(inputs["dt_head_b"], np.float32)
    opw = np.asarray(inputs["out_proj_w"], np.float32)
    D = np.asarray(inputs["D"], np.float32)

    hTp = np.zeros((DM, PL), np.float32)
    hTp[:, 3:3 + L] = h[b].T
    hTp = hTp.astype(BF)

    # tap-scaled in_proj x-weights: W_xj[d, :] = cw[d, j] * ipw[d, :]
    wxj = np.empty((128, 4, NK, NG, 128), np.float32)
    base = ipw[ch]                       # [DCH, DM]
    cwj = cw[ch]                         # [DCH, 4]
    for jj in range(4):
        wj = (base * cwj[:, jj:jj + 1]).T       # [DM, DCH]
        wxj[:, jj] = wj.reshape(NK, 128, NG, 128).transpose(1, 0, 2, 3)
    wx4 = np.ascontiguousarray(wxj.reshape(128, 4 * NK * NG * 128)).astype(BF)

    wz = np.ascontiguousarray(
        ipw[DI + j * DCH: DI + (j + 1) * DCH].T
        .reshape(NK, 128, NG, 128).transpose(1, 0, 2, 3)
        .reshape(128, NK * NG * 128)).astype(BF)
    wo = np.ascontiguousarray(
        opw[:, ch].T.reshape(NG, 128, NO, 128).transpose(1, 0, 2, 3)
        .reshape(128, NG * NO * 128)).astype(BF)
    dtwp = np.ascontiguousarray(dtw[ch].T.reshape(DTR, NG * 128)).astype(BF)
    xpwp = np.ascontiguousarray(
        xpw[:, ch].T.reshape(NG, 128, NR).transpose(1, 0, 2)
        .reshape(128, NG * NR)).astype(BF)

    dd = np.zeros((128, NG, 128), np.float32)
    for g in range(NG):
        dd[np.arange(128), g, np.arange(128)] = D[j * DCH + g * 128:
                                                  j * DCH + (g + 1) * 128]
    ddp = np.ascontiguousarray(dd.reshape(128, NG * 128)).astype(BF)

    pv = np.zeros((128, NG, 7), np.float32)
    for g in range(NG):
        rows = slice(j * DCH + g * 128, j * DCH + (g + 1) * 128)
        pv[:, g, 0:4] = cw[rows]
        pv[:, g, 4] = -dtb[rows]
        pv[:, g, 5] = D[rows]
        pv[:, g, 6] = cb[rows]
    pvec = np.ascontiguousarray(pv.reshape(128, NG * 7))

    return {
        "hTp": hTp,
        "wx4": wx4,
        "wz": wz,
        "wo": wo,
        "dtw": dtwp,
        "xpw": xpwp,
        "dd": ddp,
        "pvec": pvec,
        "ones16": np.ones((NST, 128), BF),
        "id128": np.eye(128, dtype=np.float32).astype(BF),
    }


_CACHE = {}


def _get_module(key, *args, **kw):
    if key not in _CACHE:
        _CACHE[key] = build_module(*args, **kw)
    return _CACHE[key]


def run(inputs, trace=False, trace_cores=None):
    L, DM, DI = 2048, 1024, 2048
    DCH, NST, DTR = 512, 16, 64
    nc = _get_module("full", L, DM, DI, DCH, NST, DTR, 8, True)
    in_maps = []
    for core in range(8):
        b, j = divmod(core, 4)
        in_maps.append(pack_core_inputs(inputs, b, j, L, DM, DI, DCH, NST, DTR))
    res = run_bass_kernel_spmd(
        nc, in_maps, core_ids=list(range(8)), trace=trace,
        trace_cores=trace_cores)
    outs = [r["outT"] for r in res.results]
    full = np.empty((2, L, DM), np.float32)
    for b in range(2):
        acc = outs[4 * b].astype(np.float64)
        for j in range(1, 4):
            acc = acc + outs[4 * b + j]
        full[b] = acc.T.astype(np.float32)
    return full, res


def kernel(**inputs) -> np.ndarray:
    out, _ = run(inputs, trace=False)
    return out


# revision 22
# speedup vs baseline: 1.0115x; 1.0115x over previous
"""Longhorn SSM layer on 8 Trainium2 cores.

Sharding: core (b, j) with b in {0,1}, j in {0..3} handles batch b and
d_inner channel chunk [j*512, (j+1)*512).  The x_proj contraction needs all
d_inner channels, so partial x_dbl results are AllReduced across the 4 cores
of each batch (per time-chunk).  The final out_proj partials are summed on
the host.

Layout: time axis split into H=4 chunks of 512, software-pipelined so
in_proj / collective / out_proj hide under the DVE-bound selective scan.
The depthwise conv is folded into the in_proj matmul (4 host-prescaled
weight copies accumulated at shifted time offsets).  GpSimd is kept idle
during the scan: its SBUF port is shared with the DVE 2x modes and a
running GpSimd op fully blocks them.  All products run on DVE in bf16
(2x), a_t and all copies/activations on the Scalar engine, y = sum_n s*q
and the D*x skip term accumulate on the PE via identity/diagonal matmuls.
"""

import sys

if "/opt/trn_rl_repo" not in sys.path:
    sys.path.append("/opt/trn_rl_repo")

import numpy as np
import ml_dtypes

import concourse.bacc as bacc
import concourse.bass as bass
import concourse.tile as tile
from concourse import mybir
from concourse.bass_utils import run_bass_kernel_spmd

F32 = mybir.dt.float32
BF16 = mybir.dt.bfloat16
AL = mybir.AluOpType
AF = mybir.ActivationFunctionType

BF = ml_dtypes.bfloat16


def build_module(L, DM, DI, DCH, NST, DTR, num_devices, use_collective):
    NG = DCH // 128          # d-tiles per core
    NK = DM // 128           # K-tiles for in_proj
    NO = DM // 128           # out_proj output tiles
    H = 4                    # time chunks
    C = L // H
    NR = DTR + 2 * NST       # x_proj rows
    PL = L + 4               # padded time length (3 zeros + L + 1 zero)

    nc = bacc.Bacc(
        "TRN2",
        target_bir_lowering=False,
        debug=False,
        enable_asserts=False,
        num_devices=num_devices,
    )

    # ---- I/O -------------------------------------------------------------
    hT_d = nc.dram_tensor("hTp", [DM, PL], BF16, kind="ExternalInput")
    wx4_d = nc.dram_tensor("wx4", [128, 4 * NK * NG * 128], BF16,
                           kind="ExternalInput")
    wz_d = nc.dram_tensor("wz", [128, NK * NG * 128], BF16, kind="ExternalInput")
    wo_d = nc.dram_tensor("wo", [128, NG * NO * 128], BF16, kind="ExternalInput")
    dtw_d = nc.dram_tensor("dtw", [DTR, NG * 128], BF16, kind="ExternalInput")
    xpw_d = nc.dram_tensor("xpw", [128, NG * NR], BF16, kind="ExternalInput")
    dd_d = nc.dram_tensor("dd", [128, NG * 128], BF16, kind="ExternalInput")
    pvec_d = nc.dram_tensor("pvec", [128, NG * 7], F32, kind="ExternalInput")
    ones_d = nc.dram_tensor("ones16", [NST, 128], BF16, kind="ExternalInput")
    id_d = nc.dram_tensor("id128", [128, 128], BF16, kind="ExternalInput")
    outT_d = nc.dram_tensor("outT", [DM, L], F32, kind="ExternalOutput")

    # internal DRAM (per-chunk collective buffers + broadcast rows)
    ccin_d = [nc.dram_tensor(f"ccin{h}", [NR, C], BF16, kind="Internal")
              for h in range(H)]
    ccout_d = [nc.dram_tensor(f"ccout{h}", [NR, C], BF16, kind="Internal")
               for h in range(H)]
    kbd = nc.dram_tensor("kbd", [NST, L], BF16, kind="Internal")
    qbd = nc.dram_tensor("qbd", [NST, L], BF16, kind="Internal")
    kkbd = nc.dram_tensor("kkbd", [NST, L], BF16, kind="Internal")
    cwu_i = nc.dram_tensor("cwu_i", [8, 4], F32, kind="Internal")
    cwu_o = nc.dram_tensor("cwu_o", [8, 4], F32, kind="Internal")

    groups = [[0, 1, 2, 3], [4, 5, 6, 7]] if num_devices == 8 else [[0]]

    with tile.TileContext(nc) as tc:
        with (
            tc.tile_pool(name="const", bufs=1) as constp,
            tc.tile_pool(name="persist", bufs=1) as pp,
            tc.tile_pool(name="weights", bufs=1) as wp,
            tc.tile_pool(name="hsb", bufs=2) as hp,
            tc.tile_pool(name="rows", bufs=1) as rowp,
            tc.tile_pool(name="a2", bufs=1) as a2p,
            tc.tile_pool(name="bcast", bufs=3) as bcp,
            tc.tile_pool(name="scan", bufs=4) as scp,
            tc.tile_pool(name="scana", bufs=3) as scap,
            tc.tile_pool(name="drain", bufs=2) as drp,
            tc.tile_pool(name="odr", bufs=2) as odp,
            tc.tile_pool(name="psY", bufs=1, space="PSUM") as psYp,
            tc.tile_pool(name="ps", bufs=2, space="PSUM") as psp,
            tc.tile_pool(name="pdk", bufs=2, space="PSUM") as pdkp,
        ):
            # warmup collective: absorbs channel-setup latency off AR(0)'s
            # critical path while the weight DMAs stream in
            if use_collective:
                nc.gpsimd.collective_compute(
                    "AllReduce", AL.add, replica_groups=groups,
                    ins=[cwu_i.ap()], outs=[cwu_o.ap()])

            # chunk-0 activations first, then weights ordered so chunk 0's
            # in_proj can start ASAP
            hsb0 = hp.tile([128, NK, C + 3], BF16, name="hsb", tag="hsb")
            for k in range(NK):
                nc.sync.dma_start(
                    hsb0[:, k], hT_d.ap()[k * 128:(k + 1) * 128, 0:C + 3])
            wx_sb = wp.tile([128, 4, NK, NG, 128], BF16)
            wz_sb = wp.tile([128, NK, NG, 128], BF16)
            for g in range(NG):
                nc.sync.dma_start(
                    wx_sb[:, :, :, g, :],
                    wx4_d.ap().rearrange("p (j k g m) -> p j k g m",
                                         j=4, k=NK, g=NG)[:, :, :, g, :])
                nc.sync.dma_start(
                    wz_sb[:, :, g, :],
                    wz_d.ap().rearrange("p (k g m) -> p k g m",
                                        k=NK, g=NG)[:, :, g, :])
            pvec = constp.tile([128, NG, 7], F32)
            nc.sync.dma_start(pvec, pvec_d.ap().rearrange("p (g c) -> p g c", g=NG))
            xpw_sb = wp.tile([128, NG, NR], BF16)
            nc.sync.dma_start(
                xpw_sb, xpw_d.ap().rearrange("p (g r) -> p g r", g=NG))
            dtw_sb = wp.tile([DTR, NG, 128], BF16)
            nc.sync.dma_start(
                dtw_sb, dtw_d.ap().rearrange("p (g m) -> p g m", g=NG))
            ones_sb = constp.tile([NST, 128], BF16)
            nc.sync.dma_start(ones_sb, ones_d.ap())
            id_sb = constp.tile([128, 128], BF16)
            nc.sync.dma_start(id_sb, id_d.ap())
            dd_sb = constp.tile([128, NG, 128], BF16)
            nc.sync.dma_start(dd_sb, dd_d.ap().rearrange("p (g m) -> p g m", g=NG))
            wo_sb = wp.tile([128, NG, NO, 128], BF16)
            nc.sync.dma_start(
                wo_sb, wo_d.ap().rearrange("p (g o m) -> p g o m", g=NG, o=NO))

            # persistent per-core state (bf16, 16KB/partition each)
            xs = pp.tile([128, NG, L], BF16)      # silu(conv(x))
            dtvb = pp.tile([128, NG, L], BF16)
            ub = pp.tile([128, NG, L], BF16)      # x*dtv
            silz = pp.tile([128, NG, L], BF16)    # silu(z)
            ygb = pp.tile([128, NG, L], BF16)     # gated y
            sc = pp.tile([128, NG * NST], F32)    # scan carry

            Y = psYp.tile([128, NG * C], F32)     # 4 PSUM banks

            # ---------------- emission helpers ---------------------------
            def emit_inproj_g(h, hsb, g):
                R = slice(h * C, (h + 1) * C)
                px = psp.tile([128, C], F32, name="px", tag="ps")
                for j in range(4):
                    for k in range(NK):
                        nc.tensor.matmul(px, wx_sb[:, j, k, g, :],
                                         hsb[:, k, j:j + C],
                                         start=(j == 0 and k == 0),
                                         stop=(j == 3 and k == NK - 1))
                nc.scalar.activation(xs[:, g, R], px, AF.Silu,
                                     bias=pvec[:, g, 6:7], scale=1.0)
                pz = psp.tile([128, C], F32, name="pz", tag="ps")
                for k in range(NK):
                    nc.tensor.matmul(pz, wz_sb[:, k, g, :], hsb[:, k, 3:3 + C],
                                     start=(k == 0), stop=(k == NK - 1))
                nc.scalar.activation(silz[:, g, R], pz, AF.Silu)

            def emit_xproj(h):
                R = slice(h * C, (h + 1) * C)
                pX = psp.tile([128, C], F32, name="pX", tag="ps")
                for g in range(NG):
                    nc.tensor.matmul(pX[0:NR, :], xpw_sb[:, g, :], xs[:, g, R],
                                     start=(g == 0), stop=(g == NG - 1))
                xdp = rowp.tile([NR, C], BF16, name="xdp", tag="xdp")
                nc.scalar.copy(xdp, pX[0:NR, :])
                nc.sync.dma_start(ccin_d[h].ap(), xdp)

            def emit_collective(h):
                if use_collective:
                    nc.gpsimd.collective_compute(
                        "AllReduce", AL.add, replica_groups=groups,
                        ins=[ccin_d[h].ap()], outs=[ccout_d[h].ap()])
                else:
                    nc.sync.dma_start(ccout_d[h].ap(), ccin_d[h].ap())

            def emit_a2(h):
                R = slice(h * C, (h + 1) * C)
                dtl_b = rowp.tile([DTR, C], BF16, name="dtlb", tag="dtlb")
                nc.sync.dma_start(dtl_b, ccout_d[h].ap()[0:DTR, :])
                kb16 = rowp.tile([NST, C], BF16, name="kb16", tag="kb16")
                nc.sync.dma_start(kb16, ccout_d[h].ap()[DTR:DTR + NST, :])
                qb16 = rowp.tile([NST, C], BF16, name="qb16", tag="qb16")
                nc.sync.dma_start(qb16, ccout_d[h].ap()[DTR + NST:NR, :])
                kkb = rowp.tile([NST, C], BF16, name="kkb", tag="kkb")
                nc.vector.tensor_tensor(kkb, kb16, kb16, op=AL.mult)
                nc.sync.dma_start(kbd.ap()[:, R], kb16)
                nc.sync.dma_start(qbd.ap()[:, R], qb16)
                nc.sync.dma_start(kkbd.ap()[:, R], kkb)

                psK = pdkp.tile([128, C], F32, name="psK", tag="pdk")
                nc.tensor.matmul(psK, ones_sb, kkb, start=True, stop=True)
                skt = a2p.tile([128, C], F32, name="skt", tag="skt")
                nc.scalar.activation(skt, psK, AF.Identity, bias=1.0)
                for g in range(NG):
                    pD = pdkp.tile([128, C], F32, name="pD", tag="pdk")
                    nc.tensor.matmul(pD, dtw_sb[:, g, :], dtl_b,
                                     start=True, stop=True)
                    eg = a2p.tile([128, C], F32, name="eg", tag="eg")
                    nc.scalar.activation(eg, pD, AF.Exp,
                                         bias=pvec[:, g, 4:5], scale=-1.0)
                    den = a2p.tile([128, C], F32, name="den", tag="den")
                    nc.vector.tensor_tensor(den, eg, skt, op=AL.add)
                    dtv = a2p.tile([128, C], F32, name="dtvf", tag="dtvf")
                    nc.vector.reciprocal_approx_fast(out=dtv, in_=den)
                    nc.scalar.copy(dtvb[:, g, R], dtv)
                    nc.vector.tensor_tensor(ub[:, g, R], xs[:, g, R],
                                            dtvb[:, g, R], op=AL.mult)

            def emit_bcast(h, n):
                R = slice(h * C, (h + 1) * C)
                kkb_t = bcp.tile([128, C], BF16, name="kkb_t", tag="kkb")
                nc.sync.dma_start(
                    kkb_t, kkbd.ap()[n:n + 1, R].broadcast_to([128, C]))
                kb_t = bcp.tile([128, C], BF16, name="kb_t", tag="kb")
                nc.sync.dma_start(
                    kb_t, kbd.ap()[n:n + 1, R].broadcast_to([128, C]))
                qb_t = bcp.tile([128, C], BF16, name="qb_t", tag="qb")
                nc.sync.dma_start(
                    qb_t, qbd.ap()[n:n + 1, R].broadcast_to([128, C]))
                return kkb_t, kb_t, qb_t

            def emit_products(h, n, g, kkb_t, kb_t):
                # stage 1 of a scan iteration: c, a, b (lookahead-emitted so
                # ACT's a_t computes during the previous scan instruction)
                R = slice(h * C, (h + 1) * C)
                c_t = scp.tile([128, C], BF16, name="c_t", tag="c")
                nc.vector.tensor_tensor(c_t, dtvb[:, g, R], kkb_t, op=AL.mult)
                a_t = scap.tile([128, C], F32, name="a_t", tag="a")
                nc.scalar.activation(a_t, c_t, AF.Identity, bias=1.0, scale=-1.0)
                b_t = scp.tile([128, C], BF16, name="b_t", tag="b")
                nc.vector.tensor_tensor(b_t, ub[:, g, R], kb_t, op=AL.mult)
                return a_t, b_t

            def emit_scan_tail(h, n, g, a_t, b_t, qb_t):
                col = g * NST + n
                s_t = scp.tile([128, C], BF16, name="s_t", tag="s")
                init = 0.0 if h == 0 else sc[:, col:col + 1]
                nc.vector.tensor_tensor_scan(
                    s_t, a_t, b_t, init, op0=AL.mult, op1=AL.add)
                if h < H - 1:
                    nc.scalar.copy(sc[:, col:col + 1], s_t[:, C - 1:C])
                p_t = scp.tile([128, C], BF16, name="p_t", tag="p")
                nc.vector.tensor_tensor(p_t, s_t, qb_t, op=AL.mult)
                nc.tensor.matmul(Y[:, g * C:(g + 1) * C], id_sb, p_t,
                                 start=(n == 0), stop=False)

            def emit_drain(h):
                R = slice(h * C, (h + 1) * C)
                for g in range(NG):
                    # D*x skip term closes the Y accumulation group
                    nc.tensor.matmul(Y[:, g * C:(g + 1) * C], dd_sb[:, g, :],
                                     xs[:, g, R], start=False, stop=True)
                    yb = drp.tile([128, C], BF16, name="yb", tag="yb")
                    nc.scalar.copy(yb, Y[:, g * C:(g + 1) * C])
                    nc.vector.tensor_tensor(ygb[:, g, R], yb, silz[:, g, R],
                                            op=AL.mult)

            def emit_outproj(h):
                R = slice(h * C, (h + 1) * C)
                for o in range(NO):
                    po = psp.tile([128, C], F32, name="po", tag="ps")
                    for g in range(NG):
                        nc.tensor.matmul(po, wo_sb[:, g, o, :], ygb[:, g, R],
                                         start=(g == 0), stop=(g == NG - 1))
                    ot = odp.tile([128, C], F32, name="ot", tag="ot")
                    nc.scalar.copy(ot, po)
                    nc.sync.dma_start(outT_d.ap()[o * 128:(o + 1) * 128, R], ot)

            def load_hsb(h):
                t = hp.tile([128, NK, C + 3], BF16, name="hsb", tag="hsb")
                for k in range(NK):
                    nc.sync.dma_start(
                        t[:, k], hT_d.ap()[k * 128:(k + 1) * 128,
                                           h * C:h * C + C + 3])
                return t

            # ---------------- chunk 0 prologue ---------------------------
            for g in range(NG):
                emit_inproj_g(0, hsb0, g)
            emit_xproj(0)
            emit_collective(0)
            emit_a2(0)

            # ---------------- pipelined chunks ----------------------------
            # one-iteration lookahead: products (c/a/b) for iter i+1 are
            # emitted before scan_tail(i), so the ACT a_t overlaps the scan
            prev = None
            hsb_n = [None] * (H + 1)
            bc = None
            for h in range(H):
                for n in range(NST):
                    bc_new = emit_bcast(h, n)
                    for g in range(NG):
                        if g == 0:
                            bc = bc_new
                        a_t, b_t = emit_products(h, n, g, bc[0], bc[1])
                        if prev is not None:
                            emit_scan_tail(*prev)
                            ph, pn, pg = prev[0], prev[1], prev[2]
                            if pn == NST - 1 and pg == NG - 1:
                                emit_drain(ph)
                                emit_outproj(ph)
                        prev = (h, n, g, a_t, b_t, bc[2])
                    # interleave next chunk's prep into the scan
                    nxt = h + 1
                    if nxt < H:
                        if n == 0:
                            hsb_n[nxt] = load_hsb(nxt)
                        elif 1 <= n <= 4:
                            emit_inproj_g(nxt, hsb_n[nxt], n - 1)
                        elif n == 5:
                            emit_xproj(nxt)
                        elif n == 6:
                            emit_collective(nxt)
                        elif n == 11:
                            emit_a2(nxt)
            emit_scan_tail(*prev)
            emit_drain(H - 1)
            emit_outproj(H - 1)

    nc.compile()
    return nc


# ----------------------------------------------------------------------------
# host-side packing
# ----------------------------------------------------------------------------

def pack_core_inputs(inputs, b, j, L, DM, DI, DCH, NST, DTR):
    NG = DCH // 128
    NK = DM // 128
    NO = DM // 128
    NR = DTR + 2 * NST
    PL = L + 4
    ch = slice(j * DCH, (j + 1) * DCH)

    h = np.ascontiguousarray(np.asarray(inputs["hidden_states"], np.float32))
    ipw = np.asarray(inputs["in_proj_w"], np.float32)
    cw = np.asarray(inputs["conv_w"], np.float32).reshape(DI, 4)
    cb = np.asarray(inputs["conv_b"], np.float32)
    xpw = np.asarray(inputs["x_proj_w"], np.float32)
    dtw = np.asarray(inputs["dt_head_w"], np.float32)
    dtb = np.asarray(inputs["dt_head_b"], np.float32)
    opw = np.asarray(inputs["out_proj_w"], np.float32)
    D = np.asarray(inputs["D"], np.float32)

    hTp = np.zeros((DM, PL), np.float32)
    hTp[:, 3:3 + L] = h[b].T
    hTp = hTp.astype(BF)

    # tap-scaled in_proj x-weights: W_xj[d, :] = cw[d, j] * ipw[d, :]
    wxj = np.empty((128, 4, NK, NG, 128), np.float32)
    base = ipw[ch]                       # [DCH, DM]
    cwj = cw[ch]                         # [DCH, 4]
    for jj in range(4):
        wj = (base * cwj[:, jj:jj + 1]).T       # [DM, DCH]
        wxj[:, jj] = wj.reshape(NK, 128, NG, 128).transpose(1, 0, 2, 3)
    wx4 = np.ascontiguousarray(wxj.reshape(128, 4 * NK * NG * 128)).astype(BF)

    wz = np.ascontiguousarray(
        ipw[DI + j * DCH: DI + (j + 1) * DCH].T
        .reshape(NK, 128, NG, 128).transpose(1, 0, 2, 3)
        .reshape(128, NK * NG * 128)).astype(BF)
    wo = np.ascontiguousarray(
        opw[:, ch].T.reshape(NG, 128, NO, 128).transpose(1, 0, 2, 3)
        .reshape(128, NG * NO * 128)).astype(BF)
    dtwp = np.ascontiguousarray(dtw[ch].T.reshape(DTR, NG * 128)).astype(BF)
    xpwp = np.ascontiguousarray(
        xpw[:, ch].T.reshape(NG, 128, NR).transpose(1, 0, 2)
        .reshape(128, NG * NR)).astype(BF)

    dd = np.zeros((128, NG, 128), np.float32)
    for g in range(NG):
        dd[np.arange(128), g, np.arange(128)] = D[j * DCH + g * 128:
                                                  j * DCH + (g + 1) * 128]
    ddp = np.ascontiguousarray(dd.reshape(128, NG * 128)).astype(BF)

    pv = np.zeros((128, NG, 7), np.float32)
    for g in range(NG):
        rows = slice(j * DCH + g * 128, j * DCH + (g + 1) * 128)
        pv[:, g, 0:4] = cw[rows]
        pv[:, g, 4] = -dtb[rows]
        pv[:, g, 5] = D[rows]
        pv[:, g, 6] = cb[rows]
    pvec = np.ascontiguousarray(pv.reshape(128, NG * 7))

    return {
        "hTp": hTp,
        "wx4": wx4,
        "wz": wz,
        "wo": wo,
        "dtw": dtwp,
        "xpw": xpwp,
        "dd": ddp,
        "pvec": pvec,
        "ones16": np.ones((NST, 128), BF),
        "id128": np.eye(128, dtype=np.float32).astype(BF),
    }


_CACHE = {}


def _get_module(key, *args, **kw):
    if key not in _CACHE:
        _CACHE[key] = build_module(*args, **kw)
    return _CACHE[key]


def run(inputs, trace=False, trace_cores=None):
    L, DM, DI = 2048, 1024, 2048
    DCH, NST, DTR = 512, 16, 64
    nc = _get_module("full", L, DM, DI, DCH, NST, DTR, 8, True)
    in_maps = []
    for core in range(8):
        b, j = divmod(core, 4)
        in_maps.append(pack_core_inputs(inputs, b, j, L, DM, DI, DCH, NST, DTR))
    res = run_bass_kernel_spmd(
        nc, in_maps, core_ids=list(range(8)), trace=trace,
        trace_cores=trace_cores)
    outs = [r["outT"] for r in res.results]
    full = np.empty((2, L, DM), np.float32)
    for b in range(2):
        acc = outs[4 * b].astype(np.float64)
        for j in range(1, 4):
            acc = acc + outs[4 * b + j]
        full[b] = acc.T.astype(np.float32)
    return full, res


def kernel(**inputs) -> np.ndarray:
    out, _ = run(inputs, trace=False)
    return out


# revision 23
# speedup vs baseline: 1.0290x; 1.0173x over previous
"""Longhorn SSM layer on 8 Trainium2 cores.

Sharding: core (b, j) with b in {0,1}, j in {0..3} handles batch b and
d_inner channel chunk [j*512, (j+1)*512).  The x_proj contraction needs all
d_inner channels, so partial x_dbl results are AllReduced across the 4 cores
of each batch (per time-chunk).  The final out_proj partials are summed on
the host.

Layout: time axis split into H=4 chunks of 512, software-pipelined so
in_proj / collective / out_proj hide under the DVE-bound selective scan.
The depthwise conv is folded into the in_proj matmul (4 host-prescaled
weight copies accumulated at shifted time offsets).  GpSimd is kept idle
during the scan: its SBUF port is shared with the DVE 2x modes and a
running GpSimd op fully blocks them.  All products run on DVE in bf16
(2x), a_t and all copies/activations on the Scalar engine, y = sum_n s*q
and the D*x skip term accumulate on the PE via identity/diagonal matmuls.
"""

import sys

if "/opt/trn_rl_repo" not in sys.path:
    sys.path.append("/opt/trn_rl_repo")

import numpy as np
import ml_dtypes

import concourse.bacc as bacc
import concourse.bass as bass
import concourse.tile as tile
from concourse import mybir
from concourse.bass_utils import run_bass_kernel_spmd

F32 = mybir.dt.float32
BF16 = mybir.dt.bfloat16
AL = mybir.AluOpType
AF = mybir.ActivationFunctionType

BF = ml_dtypes.bfloat16


def build_module(L, DM, DI, DCH, NST, DTR, num_devices, use_collective):
    NG = DCH // 128          # d-tiles per core
    NK = DM // 128           # K-tiles for in_proj
    NO = DM // 128           # out_proj output tiles
    H = 4                    # time chunks
    C = L // H
    NR = DTR + 2 * NST       # x_proj rows
    PL = L + 4               # padded time length (3 zeros + L + 1 zero)

    nc = bacc.Bacc(
        "TRN2",
        target_bir_lowering=False,
        debug=False,
        enable_asserts=False,
        num_devices=num_devices,
    )

    # ---- I/O -------------------------------------------------------------
    hT_d = nc.dram_tensor("hTp", [DM, PL], BF16, kind="ExternalInput")
    wx4_d = nc.dram_tensor("wx4", [128, 4 * NK * NG * 128], BF16,
                           kind="ExternalInput")
    wz_d = nc.dram_tensor("wz", [128, NK * NG * 128], BF16, kind="ExternalInput")
    wo_d = nc.dram_tensor("wo", [128, NG * NO * 128], BF16, kind="ExternalInput")
    dtw_d = nc.dram_tensor("dtw", [DTR, NG * 128], BF16, kind="ExternalInput")
    xpw_d = nc.dram_tensor("xpw", [128, NG * NR], BF16, kind="ExternalInput")
    dd_d = nc.dram_tensor("dd", [128, NG * 128], BF16, kind="ExternalInput")
    pvec_d = nc.dram_tensor("pvec", [128, NG * 7], F32, kind="ExternalInput")
    ones_d = nc.dram_tensor("ones16", [NST, 128], BF16, kind="ExternalInput")
    id_d = nc.dram_tensor("id128", [128, 128], BF16, kind="ExternalInput")
    outT_d = nc.dram_tensor("outT", [DM, L], F32, kind="ExternalOutput")

    # internal DRAM (per-chunk collective buffers + broadcast rows)
    ccin_d = [nc.dram_tensor(f"ccin{h}", [NR, C], BF16, kind="Internal")
              for h in range(H)]
    ccout_d = [nc.dram_tensor(f"ccout{h}", [NR, C], BF16, kind="Internal")
               for h in range(H)]
    kbd = nc.dram_tensor("kbd", [NST, L], BF16, kind="Internal")
    qbd = nc.dram_tensor("qbd", [NST, L], BF16, kind="Internal")
    kkbd = nc.dram_tensor("kkbd", [NST, L], BF16, kind="Internal")
    cwu_i = nc.dram_tensor("cwu_i", [8, 4], F32, kind="Internal")
    cwu_o = nc.dram_tensor("cwu_o", [8, 4], F32, kind="Internal")

    groups = [[0, 1, 2, 3], [4, 5, 6, 7]] if num_devices == 8 else [[0]]

    with tile.TileContext(nc) as tc:
        with (
            tc.tile_pool(name="const", bufs=1) as constp,
            tc.tile_pool(name="persist", bufs=1) as pp,
            tc.tile_pool(name="weights", bufs=1) as wp,
            tc.tile_pool(name="hsb", bufs=2) as hp,
            tc.tile_pool(name="rows", bufs=1) as rowp,
            tc.tile_pool(name="a2", bufs=1) as a2p,
            tc.tile_pool(name="bcast", bufs=3) as bcp,
            tc.tile_pool(name="scan", bufs=5) as scp,
            tc.tile_pool(name="scana", bufs=3) as scap,
            tc.tile_pool(name="drain", bufs=2) as drp,
            tc.tile_pool(name="odr", bufs=2) as odp,
            tc.tile_pool(name="psY", bufs=1, space="PSUM") as psYp,
            tc.tile_pool(name="ps", bufs=2, space="PSUM") as psp,
            tc.tile_pool(name="pdk", bufs=2, space="PSUM") as pdkp,
        ):
            # warmup collective: absorbs channel-setup latency off AR(0)'s
            # critical path while the weight DMAs stream in
            if use_collective:
                nc.gpsimd.collective_compute(
                    "AllReduce", AL.add, replica_groups=groups,
                    ins=[cwu_i.ap()], outs=[cwu_o.ap()])

            # chunk-0 activations first, then weights ordered so chunk 0's
            # in_proj can start ASAP
            hsb0 = hp.tile([128, NK, C + 3], BF16, name="hsb", tag="hsb")
            for k in range(NK):
                nc.sync.dma_start(
                    hsb0[:, k], hT_d.ap()[k * 128:(k + 1) * 128, 0:C + 3])
            wx_sb = wp.tile([128, 4, NK, NG, 128], BF16)
            wz_sb = wp.tile([128, NK, NG, 128], BF16)
            for g in range(NG):
                nc.sync.dma_start(
                    wx_sb[:, :, :, g, :],
                    wx4_d.ap().rearrange("p (j k g m) -> p j k g m",
                                         j=4, k=NK, g=NG)[:, :, :, g, :])
                nc.sync.dma_start(
                    wz_sb[:, :, g, :],
                    wz_d.ap().rearrange("p (k g m) -> p k g m",
                                        k=NK, g=NG)[:, :, g, :])
            pvec = constp.tile([128, NG, 7], F32)
            nc.sync.dma_start(pvec, pvec_d.ap().rearrange("p (g c) -> p g c", g=NG))
            xpw_sb = wp.tile([128, NG, NR], BF16)
            nc.sync.dma_start(
                xpw_sb, xpw_d.ap().rearrange("p (g r) -> p g r", g=NG))
            dtw_sb = wp.tile([DTR, NG, 128], BF16)
            nc.sync.dma_start(
                dtw_sb, dtw_d.ap().rearrange("p (g m) -> p g m", g=NG))
            ones_sb = constp.tile([NST, 128], BF16)
            nc.sync.dma_start(ones_sb, ones_d.ap())
            id_sb = constp.tile([128, 128], BF16)
            nc.sync.dma_start(id_sb, id_d.ap())
            dd_sb = constp.tile([128, NG, 128], BF16)
            nc.sync.dma_start(dd_sb, dd_d.ap().rearrange("p (g m) -> p g m", g=NG))
            wo_sb = wp.tile([128, NG, NO, 128], BF16)
            nc.sync.dma_start(
                wo_sb, wo_d.ap().rearrange("p (g o m) -> p g o m", g=NG, o=NO))

            # persistent per-core state (bf16, 16KB/partition each)
            xs = pp.tile([128, NG, L], BF16)      # silu(conv(x))
            dtvb = pp.tile([128, NG, L], BF16)
            ub = pp.tile([128, NG, L], BF16)      # x*dtv
            silz = pp.tile([128, NG, L], BF16)    # silu(z)
            ygb = pp.tile([128, NG, L], BF16)     # gated y
            sc = pp.tile([128, NG * NST], F32)    # scan carry

            Y = psYp.tile([128, NG * C], F32)     # 4 PSUM banks

            # ---------------- emission helpers ---------------------------
            def emit_inproj_g(h, hsb, g):
                R = slice(h * C, (h + 1) * C)
                px = psp.tile([128, C], F32, name="px", tag="ps")
                for j in range(4):
                    for k in range(NK):
                        nc.tensor.matmul(px, wx_sb[:, j, k, g, :],
                                         hsb[:, k, j:j + C],
                                         start=(j == 0 and k == 0),
                                         stop=(j == 3 and k == NK - 1))
                nc.scalar.activation(xs[:, g, R], px, AF.Silu,
                                     bias=pvec[:, g, 6:7], scale=1.0)
                pz = psp.tile([128, C], F32, name="pz", tag="ps")
                for k in range(NK):
                    nc.tensor.matmul(pz, wz_sb[:, k, g, :], hsb[:, k, 3:3 + C],
                                     start=(k == 0), stop=(k == NK - 1))
                nc.scalar.activation(silz[:, g, R], pz, AF.Silu)

            def emit_xproj(h):
                R = slice(h * C, (h + 1) * C)
                pX = psp.tile([128, C], F32, name="pX", tag="ps")
                for g in range(NG):
                    nc.tensor.matmul(pX[0:NR, :], xpw_sb[:, g, :], xs[:, g, R],
                                     start=(g == 0), stop=(g == NG - 1))
                xdp = rowp.tile([NR, C], BF16, name="xdp", tag="xdp")
                nc.scalar.copy(xdp, pX[0:NR, :])
                nc.sync.dma_start(ccin_d[h].ap(), xdp)

            def emit_collective(h):
                if use_collective:
                    nc.gpsimd.collective_compute(
                        "AllReduce", AL.add, replica_groups=groups,
                        ins=[ccin_d[h].ap()], outs=[ccout_d[h].ap()])
                else:
                    nc.sync.dma_start(ccout_d[h].ap(), ccin_d[h].ap())

            def emit_a2(h):
                R = slice(h * C, (h + 1) * C)
                dtl_b = rowp.tile([DTR, C], BF16, name="dtlb", tag="dtlb")
                nc.sync.dma_start(dtl_b, ccout_d[h].ap()[0:DTR, :])
                kb16 = rowp.tile([NST, C], BF16, name="kb16", tag="kb16")
                nc.sync.dma_start(kb16, ccout_d[h].ap()[DTR:DTR + NST, :])
                qb16 = rowp.tile([NST, C], BF16, name="qb16", tag="qb16")
                nc.sync.dma_start(qb16, ccout_d[h].ap()[DTR + NST:NR, :])
                kkb = rowp.tile([NST, C], BF16, name="kkb", tag="kkb")
                nc.vector.tensor_tensor(kkb, kb16, kb16, op=AL.mult)
                nc.sync.dma_start(kbd.ap()[:, R], kb16)
                nc.sync.dma_start(qbd.ap()[:, R], qb16)
                nc.sync.dma_start(kkbd.ap()[:, R], kkb)

                psK = pdkp.tile([128, C], F32, name="psK", tag="pdk")
                nc.tensor.matmul(psK, ones_sb, kkb, start=True, stop=True)
                skt = a2p.tile([128, C], F32, name="skt", tag="skt")
                nc.scalar.activation(skt, psK, AF.Identity, bias=1.0)
                for g in range(NG):
                    pD = pdkp.tile([128, C], F32, name="pD", tag="pdk")
                    nc.tensor.matmul(pD, dtw_sb[:, g, :], dtl_b,
                                     start=True, stop=True)
                    eg = a2p.tile([128, C], F32, name="eg", tag="eg")
                    nc.scalar.activation(eg, pD, AF.Exp,
                                         bias=pvec[:, g, 4:5], scale=-1.0)
                    den = a2p.tile([128, C], F32, name="den", tag="den")
                    nc.vector.tensor_tensor(den, eg, skt, op=AL.add)
                    dtv = a2p.tile([128, C], F32, name="dtvf", tag="dtvf")
                    nc.vector.reciprocal_approx_fast(out=dtv, in_=den)
                    nc.scalar.copy(dtvb[:, g, R], dtv)
                    nc.vector.tensor_tensor(ub[:, g, R], xs[:, g, R],
                                            dtvb[:, g, R], op=AL.mult)

            def emit_bcast(h, n):
                R = slice(h * C, (h + 1) * C)
                kkb_t = bcp.tile([128, C], BF16, name="kkb_t", tag="kkb")
                nc.sync.dma_start(
                    kkb_t, kkbd.ap()[n:n + 1, R].broadcast_to([128, C]))
                kb_t = bcp.tile([128, C], BF16, name="kb_t", tag="kb")
                nc.sync.dma_start(
                    kb_t, kbd.ap()[n:n + 1, R].broadcast_to([128, C]))
                qb_t = bcp.tile([128, C], BF16, name="qb_t", tag="qb")
                nc.sync.dma_start(
                    qb_t, qbd.ap()[n:n + 1, R].broadcast_to([128, C]))
                return kkb_t, kb_t, qb_t

            def emit_products(h, n, g, kkb_t, kb_t):
                # stage 1 of a scan iteration: c, a, b (lookahead-emitted so
                # ACT's a_t computes during the previous scan instruction)
                R = slice(h * C, (h + 1) * C)
                c_t = scp.tile([128, C], BF16, name="c_t", tag="c")
                nc.vector.tensor_tensor(c_t, dtvb[:, g, R], kkb_t, op=AL.mult)
                a_t = scap.tile([128, C], F32, name="a_t", tag="a")
                nc.scalar.activation(a_t, c_t, AF.Identity, bias=1.0, scale=-1.0)
                b_t = scp.tile([128, C], BF16, name="b_t", tag="b")
                nc.vector.tensor_tensor(b_t, ub[:, g, R], kb_t, op=AL.mult)
                return a_t, b_t

            def emit_scan_tail(h, n, g, a_t, b_t, qb_t):
                col = g * NST + n
                s_t = scp.tile([128, C], BF16, name="s_t", tag="s")
                init = 0.0 if h == 0 else sc[:, col:col + 1]
                nc.vector.tensor_tensor_scan(
                    s_t, a_t, b_t, init, op0=AL.mult, op1=AL.add)
                if h < H - 1:
                    nc.scalar.copy(sc[:, col:col + 1], s_t[:, C - 1:C])
                p_t = scp.tile([128, C], BF16, name="p_t", tag="p")
                nc.vector.tensor_tensor(p_t, s_t, qb_t, op=AL.mult)
                nc.tensor.matmul(Y[:, g * C:(g + 1) * C], id_sb, p_t,
                                 start=(n == 0), stop=False)

            def emit_drain(h):
                R = slice(h * C, (h + 1) * C)
                for g in range(NG):
                    # D*x skip term closes the Y accumulation group
                    nc.tensor.matmul(Y[:, g * C:(g + 1) * C], dd_sb[:, g, :],
                                     xs[:, g, R], start=False, stop=True)
                    yb = drp.tile([128, C], BF16, name="yb", tag="yb")
                    nc.scalar.copy(yb, Y[:, g * C:(g + 1) * C])
                    nc.vector.tensor_tensor(ygb[:, g, R], yb, silz[:, g, R],
                                            op=AL.mult)

            def emit_outproj(h):
                R = slice(h * C, (h + 1) * C)
                for o in range(NO):
                    po = psp.tile([128, C], F32, name="po", tag="ps")
                    for g in range(NG):
                        nc.tensor.matmul(po, wo_sb[:, g, o, :], ygb[:, g, R],
                                         start=(g == 0), stop=(g == NG - 1))
                    ot = odp.tile([128, C], F32, name="ot", tag="ot")
                    nc.scalar.copy(ot, po)
                    nc.sync.dma_start(outT_d.ap()[o * 128:(o + 1) * 128, R], ot)

            def load_hsb(h):
                t = hp.tile([128, NK, C + 3], BF16, name="hsb", tag="hsb")
                for k in range(NK):
                    nc.sync.dma_start(
                        t[:, k], hT_d.ap()[k * 128:(k + 1) * 128,
                                           h * C:h * C + C + 3])
                return t

            # ---------------- chunk 0 prologue ---------------------------
            for g in range(NG):
                emit_inproj_g(0, hsb0, g)
            emit_xproj(0)
            emit_collective(0)
            emit_a2(0)

            # ---------------- pipelined chunks ----------------------------
            # one-iteration lookahead: products (c/a/b) for iter i+1 are
            # emitted before scan_tail(i), so the ACT a_t overlaps the scan
            prev = None
            hsb_n = [None] * (H + 1)
            bc = None
            for h in range(H):
                for n in range(NST):
                    bc_new = emit_bcast(h, n)
                    for g in range(NG):
                        if g == 0:
                            bc = bc_new
                        a_t, b_t = emit_products(h, n, g, bc[0], bc[1])
                        if prev is not None:
                            emit_scan_tail(*prev)
                            ph, pn, pg = prev[0], prev[1], prev[2]
                            if pn == NST - 1 and pg == NG - 1:
                                emit_drain(ph)
                                emit_outproj(ph)
                        prev = (h, n, g, a_t, b_t, bc[2])
                    # interleave next chunk's prep into the scan
                    nxt = h + 1
                    if nxt < H:
                        if n == 0:
                            hsb_n[nxt] = load_hsb(nxt)
                        elif 1 <= n <= 4:
                            emit_inproj_g(nxt, hsb_n[nxt], n - 1)
                        elif n == 5:
                            emit_xproj(nxt)
                        elif n == 6:
                            emit_collective(nxt)
                        elif n == 11:
                            emit_a2(nxt)
            emit_scan_tail(*prev)
            emit_drain(H - 1)
            emit_outproj(H - 1)

    nc.compile()
    return nc


# ----------------------------------------------------------------------------
# host-side packing
# ----------------------------------------------------------------------------

def pack_core_inputs(inputs, b, j, L, DM, DI, DCH, NST, DTR):
    NG = DCH // 128
    NK = DM // 128
    NO = DM // 128
    NR = DTR + 2 * NST
    PL = L + 4
    ch = slice(j * DCH, (j + 1) * DCH)

    h = np.ascontiguousarray(np.asarray(inputs["hidden_states"], np.float32))
    ipw = np.asarray(inputs["in_proj_w"], np.float32)
    cw = np.asarray(inputs["conv_w"], np.float32).reshape(DI, 4)
    cb = np.asarray(inputs["conv_b"], np.float32)
    xpw = np.asarray(inputs["x_proj_w"], np.float32)
    dtw = np.asarray(inputs["dt_head_w"], np.float32)
    dtb = np.asarray(inputs["dt_head_b"], np.float32)
    opw = np.asarray(inputs["out_proj_w"], np.float32)
    D = np.asarray(inputs["D"], np.float32)

    hTp = np.zeros((DM, PL), np.float32)
    hTp[:, 3:3 + L] = h[b].T
    hTp = hTp.astype(BF)

    # tap-scaled in_proj x-weights: W_xj[d, :] = cw[d, j] * ipw[d, :]
    wxj = np.empty((128, 4, NK, NG, 128), np.float32)
    base = ipw[ch]                       # [DCH, DM]
    cwj = cw[ch]                         # [DCH, 4]
    for jj in range(4):
        wj = (base * cwj[:, jj:jj + 1]).T       # [DM, DCH]
        wxj[:, jj] = wj.reshape(NK, 128, NG, 128).transpose(1, 0, 2, 3)
    wx4 = np.ascontiguousarray(wxj.reshape(128, 4 * NK * NG * 128)).astype(BF)

    wz = np.ascontiguousarray(
        ipw[DI + j * DCH: DI + (j + 1) * DCH].T
        .reshape(NK, 128, NG, 128).transpose(1, 0, 2, 3)
        .reshape(128, NK * NG * 128)).astype(BF)
    wo = np.ascontiguousarray(
        opw[:, ch].T.reshape(NG, 128, NO, 128).transpose(1, 0, 2, 3)
        .reshape(128, NG * NO * 128)).astype(BF)
    dtwp = np.ascontiguousarray(dtw[ch].T.reshape(DTR, NG * 128)).astype(BF)
    xpwp = np.ascontiguousarray(
        xpw[:, ch].T.reshape(NG, 128, NR).transpose(1, 0, 2)
        .reshape(128, NG * NR)).astype(BF)

    dd = np.zeros((128, NG, 128), np.float32)
    for g in range(NG):
        dd[np.arange(128), g, np.arange(128)] = D[j * DCH + g * 128:
                                                  j * DCH + (g + 1) * 128]
    ddp = np.ascontiguousarray(dd.reshape(128, NG * 128)).astype(BF)

    pv = np.zeros((128, NG, 7), np.float32)
    for g in range(NG):
        rows = slice(j * DCH + g * 128, j * DCH + (g + 1) * 128)
        pv[:, g, 0:4] = cw[rows]
        pv[:, g, 4] = -dtb[rows]
        pv[:, g, 5] = D[rows]
        pv[:, g, 6] = cb[rows]
    pvec = np.ascontiguousarray(pv.reshape(128, NG * 7))

    return {
        "hTp": hTp,
        "wx4": wx4,
        "wz": wz,
        "wo": wo,
        "dtw": dtwp,
        "xpw": xpwp,
        "dd": ddp,
        "pvec": pvec,
        "ones16": np.ones((NST, 128), BF),
        "id128": np.eye(128, dtype=np.float32).astype(BF),
    }


_CACHE = {}


def _get_module(key, *args, **kw):
    if key not in _CACHE:
        _CACHE[key] = build_module(*args, **kw)
    return _CACHE[key]


def run(inputs, trace=False, trace_cores=None):
    L, DM, DI = 2048, 1024, 2048
    DCH, NST, DTR = 512, 16, 64
    nc = _get_module("full", L, DM, DI, DCH, NST, DTR, 8, True)
    in_maps = []
    for core in range(8):
        b, j = divmod(core, 4)
        in_maps.append(pack_core_inputs(inputs, b, j, L, DM, DI, DCH, NST, DTR))
    res = run_bass_kernel_spmd(
        nc, in_maps, core_ids=list(range(8)), trace=trace,
        trace_cores=trace_cores)
    outs = [r["outT"] for r in res.results]
    full = np.empty((2, L, DM), np.float32)
    for b in range(2):
        acc = outs[4 * b].astype(np.float64)
        for j in range(1, 4):
            acc = acc + outs[4 * b + j]
        full[b] = acc.T.astype(np.float32)
    return full, res


def kernel(**inputs) -> np.ndarray:
    out, _ = run(inputs, trace=False)
    return out
